# revision 30
# baseline (speedup 1.0000x reference)
"""Distributed single-head attention for Trainium2 (8 NeuronCores).

Problem: B=4, S=2048, D=1024 fp32 attention:
    q = x@Wq+bq; k = x@Wk+bk; v = x@Wv+bv
    out = softmax(q k^T / sqrt(D) + mask) v

Sharding: data-parallel over (batch, query-half): core c handles batch
c//2, query rows [1024*(c%2), 1024*(c%2)+1024). Each core receives the
FULL x of its batch from the host (full_io), so there are NO on-chip
collectives at all.

Algebraic trick 1 — the K projection is eliminated:
    q_i . k_j = x_i (Wq Wk^T) x_j^T + x_j.(Wk bq) + alpha_i
where alpha_i is a per-row constant that softmax drops. With
host-precomputed M2 = Wq @ Wk^T and w2 = Wk @ bq,
    scores (up to a row constant) = TT @ x_keys^T,  TT = x_q @ M2 + w2.

Algebraic trick 2 — the V projection is eliminated:
    out = A @ (x Wv + bv) = (A @ x) Wv + bv        (rows of A sum to 1)
so the kernel computes PV' = A_norm @ x against the raw local x, then a
final per-q-chunk projection O = PV' @ Wv + bv. PE work is identical to
projecting V (the O projection replaces it 1:1) but no V data is shared.

The scores matmul (the biggest PE phase) runs in fp8e4 DoubleRow mode:
TT is evicted straight to fp8 and x arrives as a host-quantized fp8
copy, both in [128, 2, N] k-subtile-paired layout, halving the scores
instruction count. The TT projection itself is hybrid: its first 512
contraction features use fp8 DoubleRow (m28 + the query columns of x8),
the rest stay bf16. fp8 feeds ONLY the softmax logits; PV'/O stay bf16.
Measured end-to-end rel err 1.72e-2 (deterministic) vs the 2e-2 gate —
numerically validated against an exact numpy emulation of the pipeline
that matches hardware to 3 digits.

Per-core key order is "own queries first": the host permutes the key
axis so columns [0,1024) are this core's own query rows (queries read
uniformly on every core -> one SPMD program); softmax+PV are invariant
to key permutation, and the mask columns are permuted to match.

Per-core host-prepared inputs:
  xq  bf16 [512(d), 1024(q)]: own query rows of x[b], transposed,
      upper half of the d axis (the lower half comes from x8).
  x8  fp8e4 [1024(d), 2048(s)]: x[b]^T key-permuted, fp8 (scores rhs).
  xn  bf16 [2048(s), 1024(d)]: x[b], key-permuted (PV' rhs).
  m2  bf16 [1024, 1024]: Wq @ Wk^T (computed in float64; only the
      upper 512 d-rows are loaded in bf16).
  m28 fp8e4 [512, 1024]: lower half of m2 for the TT DoubleRow part.
  wv  bf16 [1024, 1024].
  w22 f32 [128, 8]: Wk@bq in chunk layout (TT per-partition bias).
  bvr bf16 [1, 1024]: V bias row (partition-broadcast on load, added
      to O during the eviction tensor_add).
  maskp f32 [1024, 2048] (only when mask is nonzero): additive mask for
      this core's q rows, key-permuted, pre-divided by SCALE.

On-chip per core, software-pipelined over 8 q-chunks (128 rows each):
  TT[d',q] = M2^T x_q + w2 (hybrid fp8/bf16), evicted to fp8 pairs
  scores[q,s] = TT^T x8   (fp8 DoubleRow, fp32 PSUM, two 1024 halves)
  e = exp(SCALE*scores (+mask)), row-sums via ScalarE accum_out
  at = one 3D xbar DMA-transpose of e (bf16)
  PV'[q,d] = at^T xn, evicted *1/rowsum to bf16, xbar-transposed
  O[q,e] = PV'T^T wv (fp32 PSUM), evicted +bv to bf16 (the host
      upcasts to f32; ~0.2% extra rounding vs the 2e-2 gate)

Emission order per iteration (PE): scores(qc+1), O(qc-1), PV'(qc); the
at-transpose of qc is issued on the scalar ring BEFORE exp(qc+1) so the
in-order ring cannot delay it behind exp's wait on the scores PSUM.
Dependency-free warmup matmuls cover the ~15us engine-boot + first-load
window so the PE p-state is ramped when real work starts, and iteration
0 (which has no O phase) gets filler matmuls to bridge its exp->at-T
latency.
"""

from contextlib import ExitStack

import numpy as np
import ml_dtypes

import concourse.tile as tile
import concourse.mybir as mybir
from concourse import bacc
from concourse.bass_utils import run_bass_kernel_spmd

BF16 = mybir.dt.bfloat16
F32 = mybir.dt.float32
F8 = mybir.dt.float8e4
AF = mybir.ActivationFunctionType

D = 1024  # model dim (= contraction dim)
S = 2048  # full sequence (keys)
Q = 1024  # queries per core
P = 128  # partitions
ND = D // P  # 8 d-chunks
NS = S // P  # 16 key chunks
NQ = Q // P  # 8 query chunks
SCALE = 1.0 / float(np.sqrt(np.float32(D)))

_NC_CACHE: dict[bool, bacc.Bacc] = {}


def _build(use_mask: bool) -> bacc.Bacc:
    nc = bacc.Bacc("TRN2", target_bir_lowering=False, debug=False, num_devices=8)

    xq_d = nc.dram_tensor("xq", [D // 2, Q], BF16, kind="ExternalInput")
    x8_d = nc.dram_tensor("x8", [D, S], F8, kind="ExternalInput")
    xn_d = nc.dram_tensor("xn", [S, D], BF16, kind="ExternalInput")
    m2_d = nc.dram_tensor("m2", [D, D], BF16, kind="ExternalInput")
    m28_d = nc.dram_tensor("m28", [D // 2, D], F8, kind="ExternalInput")
    wv_d = nc.dram_tensor("wv", [D, D], BF16, kind="ExternalInput")
    w2_d = nc.dram_tensor("w22", [P, ND], F32, kind="ExternalInput")
    bv_d = nc.dram_tensor("bvr", [1, D], BF16, kind="ExternalInput")
    if use_mask:
        mask_d = nc.dram_tensor("maskp", [Q, S], F32, kind="ExternalInput")
    out_d = nc.dram_tensor("out", [Q, D], BF16, kind="ExternalOutput")

    with tile.TileContext(nc) as tc, ExitStack() as ctx:
        xa_pool = ctx.enter_context(tc.tile_pool(name="xa", bufs=ND))
        x8_pool = ctx.enter_context(tc.tile_pool(name="x8", bufs=ND // 2))
        m2_pool = ctx.enter_context(tc.tile_pool(name="m2", bufs=ND))
        wv_pool = ctx.enter_context(tc.tile_pool(name="wv", bufs=ND))
        xn_pool = ctx.enter_context(tc.tile_pool(name="xn", bufs=2))
        tt_pool = ctx.enter_context(tc.tile_pool(name="tt", bufs=ND // 2))
        const_pool = ctx.enter_context(tc.tile_pool(name="const", bufs=1))
        exp_pool = ctx.enter_context(tc.tile_pool(name="exp", bufs=2))
        at_pool = ctx.enter_context(tc.tile_pool(name="at", bufs=2))
        pvn_pool = ctx.enter_context(tc.tile_pool(name="pvn", bufs=2))
        pvt_pool = ctx.enter_context(tc.tile_pool(name="pvt", bufs=2))
        stat_pool = ctx.enter_context(tc.tile_pool(name="stat", bufs=8))
        o_pool = ctx.enter_context(tc.tile_pool(name="o", bufs=2))
        if use_mask:
            m_pool = ctx.enter_context(tc.tile_pool(name="m", bufs=2))
        psum = ctx.enter_context(tc.tile_pool(name="psum", bufs=4, space="PSUM"))

        xa = [xa_pool.tile([P, Q], BF16, tag="xa", name=f"xa{i}") for i in range(ND)]
        xa8 = [
            x8_pool.tile([P, 2 * S], F8, tag="xa8", name=f"xa8_{i}")
            for i in range(ND // 2)
        ]
        m2 = [
            m2_pool.tile([P, D], BF16, tag="m2", name=f"m2{i}")
            for i in range(ND // 2, ND)
        ]
        m28 = [
            m2_pool.tile([P, 2 * D], F8, tag="m28", name=f"m28_{i}")
            for i in range(2)
        ]
        wv = [wv_pool.tile([P, D], BF16, tag="wv", name=f"wv{i}") for i in range(ND)]
        xn = [
            xn_pool.tile([P, 8 * D], BF16, tag="xn", name=f"xn{i}") for i in range(2)
        ]
        w2_sb = const_pool.tile([P, ND], F32, tag="w2")
        bvb_sb = const_pool.tile([P, D], BF16, tag="bvb")

        # Load order is bandwidth-critical, and a DMA *issue* occupies the
        # issuing engine for ~0.6-2us (paced by the ring), so engines with
        # compute duties must not issue long load trains ahead of them.
        #   sync ring:   xq (TT rhs) -> x8 (scores rhs) -> out
        #   scalar ring: w2/bv -> m2 (TT lhsT) -> TT evicts/exp/transposes
        #   gpsimd:      xn (PV' rhs) -> wv (O rhs)   (software DGE pool)
        nc.scalar.dma_start(w2_sb[:], w2_d[:, :])
        # bv broadcast to all partitions (stride-0 DMA source) so the O bias
        # is a plain tensor_add fused into the eviction, not rank-1 matmuls
        nc.scalar.dma_start(bvb_sb[:], bv_d[0:1, :].partition_broadcast(P))
        # PE warmup: dependency-free full-contraction dummy matmuls fill the
        # engine-boot + load window so the tensor engine p-state is fully
        # ramped (and stays ramped) when TT starts
        warm_in = const_pool.tile([P, 512], BF16, tag="warmin")
        nc.vector.memset(warm_in[:], 0.001)
        warm = psum.tile([P, Q], F32, tag="ps", name="warm")
        for _ in range(26):
            nc.tensor.matmul(
                warm[:, 0:512],
                lhsT=warm_in[:, 0:P],
                rhs=warm_in[:, 0:512],
                start=True,
                stop=True,
            )
        # TT contraction d=0..3 uses fp8 (m28 + xa8 query cols); only the
        # upper half of m2/xq is needed in bf16
        for g in range(2):
            for i in range(2):
                nc.scalar.dma_start(
                    m28[g][:, i * D : (i + 1) * D],
                    m28_d[(2 * g + i) * P : (2 * g + i + 1) * P, :],
                )
        def load_x8(d):
            g, i = divmod(d, 2)
            nc.sync.dma_start(
                xa8[g][:, i * S : (i + 1) * S], x8_d[d * P : (d + 1) * P, :]
            )

        # d=0..3 of x8 feed TT's DoubleRow half, xq feeds its bf16 half,
        # d=4..7 of x8 are only needed once scores start
        for d in range(ND // 2):
            load_x8(d)
            nc.scalar.dma_start(
                m2[d][:], m2_d[(ND // 2 + d) * P : (ND // 2 + d + 1) * P, :]
            )
        for dd in range(ND // 2):
            nc.sync.dma_start(xa[dd][:], xq_d[dd * P : (dd + 1) * P, :])
        for d in range(ND // 2, ND):
            load_x8(d)
        # xn: [2048,1024] -> two [128, 8*1024] tiles, partition p holds
        # rows {k*128+p}; one strided DMA each
        xn_src = xn_d.ap().rearrange("(i k p) d -> i p k d", i=2, p=P)
        for i in range(2):
            nc.sync.dma_start(
                xn[i].rearrange("p (k d) -> p k d", k=8), xn_src[i]
            )
        for d in range(ND):
            nc.sync.dma_start(wv[d][:], wv_d[d * P : (d + 1) * P, :])

        # ---- TT[d',q] = M2^T xq + w2, evicted to fp8 k-subtile pairs ----
        tt8 = [
            tt_pool.tile([P, 2 * Q], F8, tag="tt", name=f"tt8_{i}")
            for i in range(ND // 2)
        ]
        for eb in range(2):
            pss = [
                psum.tile([P, Q], F32, tag="ps", name=f"tps{eb}_{j}")
                for j in range(4)
            ]
            for g in range(2):
                for j in range(4):
                    e = eb * 4 + j
                    lt = m28[g].rearrange("p (i e) -> p i e", i=2)[
                        :, :, e * P : (e + 1) * P
                    ]
                    for n in range(2):
                        rh = xa8[g].rearrange("p (i s) -> p i s", i=2)[
                            :, :, n * 512 : (n + 1) * 512
                        ]
                        nc.tensor.matmul(
                            pss[j][:, n * 512 : (n + 1) * 512],
                            lhsT=lt,
                            rhs=rh,
                            start=(g == 0),
                            stop=False,
                            perf_mode=mybir.MatmulPerfMode.DoubleRow,
                        )
            for dd in range(ND // 2):
                for j in range(4):
                    e = eb * 4 + j
                    for n in range(2):
                        nc.tensor.matmul(
                            pss[j][:, n * 512 : (n + 1) * 512],
                            lhsT=m2[dd][:, e * P : (e + 1) * P],
                            rhs=xa[dd][:, n * 512 : (n + 1) * 512],
                            start=False,
                            stop=(dd == ND // 2 - 1),
                        )
            for j in range(4):
                e = eb * 4 + j
                dst = tt8[e // 2][:, (e % 2) * Q : (e % 2) * Q + Q]
                if j % 2 == 0:
                    nc.scalar.activation(
                        dst, pss[j][:], AF.Identity, bias=w2_sb[:, e : e + 1]
                    )
                else:
                    nc.vector.tensor_scalar_add(dst, pss[j][:], w2_sb[:, e : e + 1])

        # ---- attention, software-pipelined over 8 q-chunks ----
        def scores_mm(qc):
            """fp8 DoubleRow scores matmuls; returns the two psum halves."""
            pss = []
            for half in range(2):
                ps = psum.tile([P, Q], F32, tag="ps", name=f"sps{qc}_{half}")
                for g in range(ND // 2):
                    lt = tt8[g].rearrange("p (i q) -> p i q", i=2)[
                        :, :, qc * P : (qc + 1) * P
                    ]
                    for n in range(2):
                        off = half * 1024 + n * 512
                        rh = xa8[g].rearrange("p (i s) -> p i s", i=2)[
                            :, :, off : off + 512
                        ]
                        nc.tensor.matmul(
                            ps[:, n * 512 : (n + 1) * 512],
                            lhsT=lt,
                            rhs=rh,
                            start=(g == 0),
                            stop=(g == ND // 2 - 1),
                            perf_mode=mybir.MatmulPerfMode.DoubleRow,
                        )
                pss.append(ps)
            return pss

        def exp_phase(qc, pss):
            """(+mask) + exp + row sums for q-chunk qc."""
            exp_sb = exp_pool.tile([P, S], BF16, tag="exp", name=f"exp{qc}")
            sums = stat_pool.tile([P, 2], F32, tag="sums", name=f"sums{qc}")
            for half in range(2):
                ps = pss[half]
                if use_mask:
                    mt = m_pool.tile([P, Q], F32, tag="m", name=f"mt{qc}_{half}")
                    nc.sync.dma_start(
                        mt[:],
                        mask_d[qc * P : (qc + 1) * P, half * 1024 : (half + 1) * 1024],
                    )
                    nc.vector.tensor_add(ps[:], ps[:], mt[:])
                nc.scalar.activation(
                    exp_sb[:, half * 1024 : (half + 1) * 1024],
                    ps[:],
                    AF.Exp,
                    scale=SCALE,
                    accum_out=sums[:, half : half + 1],
                )
            return exp_sb, sums

        def at_phase(qc, exp_sb, sums):
            """rowsum stats + one xbar transpose of exp for q-chunk qc."""
            rsum = stat_pool.tile([P, 1], F32, tag="rsum", name=f"rsum{qc}")
            nc.vector.tensor_add(rsum[:], sums[:, 0:1], sums[:, 1:2])
            rinv = stat_pool.tile([P, 1], F32, tag="rinv", name=f"rinv{qc}")
            nc.vector.reciprocal(rinv[:], rsum[:])
            at_sb = at_pool.tile([P, S], BF16, tag="at", name=f"at{qc}")
            # xbar transpose in halves: out[p, c, q] = exp[q, c*128+p]; the
            # first half only needs exp h0, so PV' k=0..7 unblocks earlier
            for hh in range(2):
                sl = slice(hh * 1024, (hh + 1) * 1024)
                nc.scalar.dma_start(
                    out=at_sb[:, sl].rearrange("p (c q) -> p c q", q=P),
                    in_=exp_sb[:, sl],
                    transpose=True,
                )
            return at_sb, rinv

        def pv_mm(qc, at_sb, rinv):
            """PV' = A @ x, evicted *1/rowsum to bf16 + transposed."""
            pv = psum.tile([P, D], F32, tag="ps", name=f"pv{qc}")
            for k in range(NS):
                for n in range(2):
                    nc.tensor.matmul(
                        pv[:, n * 512 : (n + 1) * 512],
                        lhsT=at_sb[:, k * P : (k + 1) * P],
                        rhs=xn[k // 8][
                            :, (k % 8) * 1024 + n * 512 : (k % 8) * 1024 + (n + 1) * 512
                        ],
                        start=(k == 0),
                        stop=(k == NS - 1),
                    )
            # evict+transpose in halves so O's first d-chunks unblock after
            # half the latency (the O phase follows only ~3.5us of scores)
            pvn = pvn_pool.tile([P, D], BF16, tag="pvn", name=f"pvn{qc}")
            pvt = pvt_pool.tile([P, D], BF16, tag="pvt", name=f"pvt{qc}")
            for hh in range(2):
                sl = slice(hh * 512, (hh + 1) * 512)
                nc.vector.tensor_scalar_mul(pvn[:, sl], pv[:, sl], rinv[:])
                nc.scalar.dma_start(
                    out=pvt[:, sl].rearrange("p (c q) -> p c q", q=P),
                    in_=pvn[:, sl],
                    transpose=True,
                )
            return pvt

        def o_phase(qc, pvt):
            """O = PV'_norm @ Wv + bv for q-chunk qc, then DMA out."""
            op = psum.tile([P, D], F32, tag="ps", name=f"op{qc}")
            for dc in range(ND):
                for n in range(2):
                    nc.tensor.matmul(
                        op[:, n * 512 : (n + 1) * 512],
                        lhsT=pvt[:, dc * P : (dc + 1) * P],
                        rhs=wv[dc][:, n * 512 : (n + 1) * 512],
                        start=(dc == 0),
                        stop=(dc == ND - 1),
                    )
            oo = o_pool.tile([P, D], BF16, tag="o", name=f"oo{qc}")
            if qc == NQ - 1:
                # final chunk is the kernel tail: evict+store in halves
                for hh in range(2):
                    sl = slice(hh * 512, (hh + 1) * 512)
                    nc.vector.tensor_add(oo[:, sl], op[:, sl], bvb_sb[:, sl])
                    nc.sync.dma_start(
                        out_d[qc * P : (qc + 1) * P, sl], oo[:, sl]
                    )
            else:
                nc.vector.tensor_add(oo[:], op[:], bvb_sb[:, :])
                nc.sync.dma_start(out_d[qc * P : (qc + 1) * P, :], oo[:])

        pend = exp_phase(0, scores_mm(0))
        prev_pvt = None
        for qc in range(NQ):
            if qc + 1 < NQ:
                pss = scores_mm(qc + 1)
                if prev_pvt is not None:
                    o_phase(qc - 1, prev_pvt)
                else:
                    # iteration 0 has no O phase to cover the exp->at-T
                    # latency of chunk 0; keep the PE warm with fillers
                    # (this alloc also keeps the psum ring period-4)
                    warm2 = psum.tile([P, Q], F32, tag="ps", name="warm2")
                    for _ in range(8):
                        nc.tensor.matmul(
                            warm2[:, 0:512],
                            lhsT=warm_in[:, 0:P],
                            rhs=warm_in[:, 0:512],
                            start=True,
                            stop=True,
                        )
                atr = at_phase(qc, *pend)
                pend = exp_phase(qc + 1, pss)
                prev_pvt = pv_mm(qc, *atr)
            else:
                # last chunk: no scores to pipeline, so run PV' first and
                # let O(qc-1) cover the evict+transpose latency of pvt(qc)
                atr = at_phase(qc, *pend)
                cur = pv_mm(qc, *atr)
                o_phase(qc - 1, prev_pvt)
                prev_pvt = cur
        o_phase(NQ - 1, prev_pvt)

    nc.compile()
    return nc


def _get_nc(use_mask: bool) -> bacc.Bacc:
    if use_mask not in _NC_CACHE:
        _NC_CACHE[use_mask] = _build(use_mask)
    return _NC_CACHE[use_mask]


def kernel(x, mask, Wq, bq, Wk, bk, Wv, bv):
    x = np.asarray(x, dtype=np.float32)
    mask = np.asarray(mask, dtype=np.float32)
    Wq = np.asarray(Wq, dtype=np.float32)
    bq = np.asarray(bq, dtype=np.float32)
    Wk = np.asarray(Wk, dtype=np.float32)
    bk = np.asarray(bk, dtype=np.float32)
    Wv = np.asarray(Wv, dtype=np.float32)
    bv = np.asarray(bv, dtype=np.float32)

    B = x.shape[0]
    use_mask = bool(np.any(mask))
    nc = _get_nc(use_mask)

    bf = ml_dtypes.bfloat16
    # scores(i,j) = q_i.k_j - alpha_i with M2 = Wq Wk^T, w2 = Wk bq;
    # alpha_i is a per-row constant that softmax drops.
    m2 = (Wq.astype(np.float64) @ Wk.astype(np.float64).T).astype(bf)
    m28 = m2[0 : D // 2, :].astype(ml_dtypes.float8_e4m3)
    w2 = (Wk.astype(np.float64) @ bq.astype(np.float64)).astype(np.float32)
    w22 = np.ascontiguousarray(w2.reshape(ND, P).T)
    wv_b = Wv.astype(bf)
    bvr = bv.reshape(1, D).astype(bf)

    in_maps = []
    for c in range(8):
        b, h = divmod(c, 2)
        # key order: own query rows first, then the other half
        xp = np.concatenate(
            [x[b, h * Q : (h + 1) * Q], x[b, (1 - h) * Q : (2 - h) * Q]]
        )
        xn = np.ascontiguousarray(xp).astype(bf)
        xpt = np.ascontiguousarray(xp.T)
        im = {
            "xq": np.ascontiguousarray(xpt[D // 2 :, 0:Q]).astype(bf),
            "x8": xpt.astype(ml_dtypes.float8_e4m3),
            "xn": xn,
            "m2": m2,
            "m28": m28,
            "wv": wv_b,
            "w22": w22,
            "bvr": bvr,
        }
        if use_mask:
            mrows = mask[h * Q : (h + 1) * Q]
            mperm = np.concatenate(
                [mrows[:, h * Q : (h + 1) * Q], mrows[:, (1 - h) * Q : (2 - h) * Q]],
                axis=1,
            )
            im["maskp"] = np.ascontiguousarray(mperm / np.float32(SCALE)).astype(
                np.float32
            )
        in_maps.append(im)

    res = run_bass_kernel_spmd(nc, in_maps, core_ids=list(range(8)))

    out = np.empty((B, S, D), dtype=np.float32)
    for c in range(8):
        b, h = divmod(c, 2)
        out[b, h * Q : (h + 1) * Q, :] = res.results[c]["out"].astype(np.float32)
    return out


# revision 32
# speedup vs baseline: 1.0088x; 1.0088x over previous
"""Distributed single-head attention for Trainium2 (8 NeuronCores).

Problem: B=4, S=2048, D=1024 fp32 attention:
    q = x@Wq+bq; k = x@Wk+bk; v = x@Wv+bv
    out = softmax(q k^T / sqrt(D) + mask) v

Sharding: data-parallel over (batch, query-half): core c handles batch
c//2, query rows [1024*(c%2), 1024*(c%2)+1024). Each core receives the
FULL x of its batch from the host (full_io), so there are NO on-chip
collectives at all.

Algebraic trick 1 — the K projection is eliminated:
    q_i . k_j = x_i (Wq Wk^T) x_j^T + x_j.(Wk bq) + alpha_i
where alpha_i is a per-row constant that softmax drops. With
host-precomputed M2 = Wq @ Wk^T and w2 = Wk @ bq,
    scores (up to a row constant) = TT @ x_keys^T,  TT = x_q @ M2 + w2.

Algebraic trick 2 — the V projection is eliminated:
    out = A @ (x Wv + bv) = (A @ x) Wv + bv        (rows of A sum to 1)
so the kernel computes PV' = A_norm @ x against the raw local x, then a
final per-q-chunk projection O = PV' @ Wv + bv. PE work is identical to
projecting V (the O projection replaces it 1:1) but no V data is shared.

The scores matmul (the biggest PE phase) runs in fp8e4 DoubleRow mode:
TT is evicted straight to fp8 and x arrives as a host-quantized fp8
copy, both in [128, 2, N] k-subtile-paired layout, halving the scores
instruction count. The TT projection itself is hybrid: its first 512
contraction features use fp8 DoubleRow (m28 + the query columns of x8),
the rest stay bf16. fp8 feeds ONLY the softmax logits; PV'/O stay bf16.
Measured end-to-end rel err 1.72e-2 (deterministic) vs the 2e-2 gate —
numerically validated against an exact numpy emulation of the pipeline
that matches hardware to 3 digits.

Per-core key order is "own queries first": the host permutes the key
axis so columns [0,1024) are this core's own query rows (queries read
uniformly on every core -> one SPMD program); softmax+PV are invariant
to key permutation, and the mask columns are permuted to match.

Per-core host-prepared inputs:
  xq  bf16 [512(d), 1024(q)]: own query rows of x[b], transposed,
      upper half of the d axis (the lower half comes from x8).
  x8  fp8e4 [1024(d), 2048(s)]: x[b]^T key-permuted, fp8 (scores rhs).
  xn  bf16 [2048(s), 1024(d)]: x[b], key-permuted (PV' rhs).
  m2  bf16 [1024, 1024]: Wq @ Wk^T (computed in float64; only the
      upper 512 d-rows are loaded in bf16).
  m28 fp8e4 [512, 1024]: lower half of m2 for the TT DoubleRow part.
  wv  bf16 [1024, 1024].
  w22 f32 [128, 8]: Wk@bq in chunk layout (TT per-partition bias).
  bvr bf16 [1, 1024]: V bias row (partition-broadcast on load, added
      to O during the eviction tensor_add).
  maskp f32 [1024, 2048] (only when mask is nonzero): additive mask for
      this core's q rows, key-permuted, pre-divided by SCALE.

On-chip per core, software-pipelined over 8 q-chunks (128 rows each):
  TT[d',q] = M2^T x_q + w2 (hybrid fp8/bf16), evicted to fp8 pairs
  scores[q,s] = TT^T x8   (fp8 DoubleRow, fp32 PSUM, two 1024 halves)
  e = exp(SCALE*scores (+mask)), row-sums via ScalarE accum_out
  at = one 3D xbar DMA-transpose of e (bf16)
  PV'[q,d] = at^T xn, evicted *1/rowsum to bf16, xbar-transposed
  O[q,e] = PV'T^T wv (fp32 PSUM), evicted +bv to bf16 (the host
      upcasts to f32; ~0.2% extra rounding vs the 2e-2 gate)

Emission order per iteration (PE): scores(qc+1), O(qc-1), PV'(qc); the
at-transpose of qc is issued on the scalar ring BEFORE exp(qc+1) so the
in-order ring cannot delay it behind exp's wait on the scores PSUM.
Dependency-free warmup matmuls cover the ~15us engine-boot + first-load
window so the PE p-state is ramped when real work starts, and iteration
0 (which has no O phase) gets filler matmuls to bridge its exp->at-T
latency.
"""

from contextlib import ExitStack

import numpy as np
import ml_dtypes

import concourse.tile as tile
import concourse.mybir as mybir
from concourse import bacc
from concourse.bass_utils import run_bass_kernel_spmd

BF16 = mybir.dt.bfloat16
F32 = mybir.dt.float32
F8 = mybir.dt.float8e4
AF = mybir.ActivationFunctionType

D = 1024  # model dim (= contraction dim)
S = 2048  # full sequence (keys)
Q = 1024  # queries per core
P = 128  # partitions
ND = D // P  # 8 d-chunks
NS = S // P  # 16 key chunks
NQ = Q // P  # 8 query chunks
SCALE = 1.0 / float(np.sqrt(np.float32(D)))

_NC_CACHE: dict[bool, bacc.Bacc] = {}


def _build(use_mask: bool) -> bacc.Bacc:
    nc = bacc.Bacc("TRN2", target_bir_lowering=False, debug=False, num_devices=8)

    xq_d = nc.dram_tensor("xq", [D // 2, Q], BF16, kind="ExternalInput")
    x8_d = nc.dram_tensor("x8", [D, S], F8, kind="ExternalInput")
    xn_d = nc.dram_tensor("xn", [S, D], BF16, kind="ExternalInput")
    m2_d = nc.dram_tensor("m2", [D, D], BF16, kind="ExternalInput")
    m28_d = nc.dram_tensor("m28", [D // 2, D], F8, kind="ExternalInput")
    wv_d = nc.dram_tensor("wv", [D, D], BF16, kind="ExternalInput")
    w2_d = nc.dram_tensor("w22", [P, ND], F32, kind="ExternalInput")
    bv_d = nc.dram_tensor("bvr", [1, D], BF16, kind="ExternalInput")
    if use_mask:
        mask_d = nc.dram_tensor("maskp", [Q, S], F32, kind="ExternalInput")
    out_d = nc.dram_tensor("out", [Q, D], BF16, kind="ExternalOutput")

    with tile.TileContext(nc) as tc, ExitStack() as ctx:
        xa_pool = ctx.enter_context(tc.tile_pool(name="xa", bufs=ND))
        x8_pool = ctx.enter_context(tc.tile_pool(name="x8", bufs=ND // 2))
        m2_pool = ctx.enter_context(tc.tile_pool(name="m2", bufs=ND))
        wv_pool = ctx.enter_context(tc.tile_pool(name="wv", bufs=ND))
        xn_pool = ctx.enter_context(tc.tile_pool(name="xn", bufs=2))
        tt_pool = ctx.enter_context(tc.tile_pool(name="tt", bufs=ND // 2))
        const_pool = ctx.enter_context(tc.tile_pool(name="const", bufs=1))
        exp_pool = ctx.enter_context(tc.tile_pool(name="exp", bufs=2))
        at_pool = ctx.enter_context(tc.tile_pool(name="at", bufs=2))
        pvn_pool = ctx.enter_context(tc.tile_pool(name="pvn", bufs=2))
        pvt_pool = ctx.enter_context(tc.tile_pool(name="pvt", bufs=2))
        stat_pool = ctx.enter_context(tc.tile_pool(name="stat", bufs=8))
        o_pool = ctx.enter_context(tc.tile_pool(name="o", bufs=2))
        if use_mask:
            m_pool = ctx.enter_context(tc.tile_pool(name="m", bufs=2))
        psum = ctx.enter_context(tc.tile_pool(name="psum", bufs=4, space="PSUM"))

        xa = [xa_pool.tile([P, Q], BF16, tag="xa", name=f"xa{i}") for i in range(ND)]
        xa8 = [
            x8_pool.tile([P, 2 * S], F8, tag="xa8", name=f"xa8_{i}")
            for i in range(ND // 2)
        ]
        m2 = [
            m2_pool.tile([P, D], BF16, tag="m2", name=f"m2{i}")
            for i in range(ND // 2, ND)
        ]
        m28 = [
            m2_pool.tile([P, 2 * D], F8, tag="m28", name=f"m28_{i}")
            for i in range(2)
        ]
        wv = [wv_pool.tile([P, D], BF16, tag="wv", name=f"wv{i}") for i in range(ND)]
        xn = [
            xn_pool.tile([P, 8 * D], BF16, tag="xn", name=f"xn{i}") for i in range(2)
        ]
        w2_sb = const_pool.tile([P, ND], F32, tag="w2")
        bvb_sb = const_pool.tile([P, D], BF16, tag="bvb")

        # Load order is bandwidth-critical, and a DMA *issue* occupies the
        # issuing engine for ~0.6-2us (paced by the ring), so engines with
        # compute duties must not issue long load trains ahead of them.
        #   sync ring:   xq (TT rhs) -> x8 (scores rhs) -> out
        #   scalar ring: w2/bv -> m2 (TT lhsT) -> TT evicts/exp/transposes
        #   gpsimd:      xn (PV' rhs) -> wv (O rhs)   (software DGE pool)
        # PE warmup: dependency-free full-contraction dummy matmuls fill the
        # engine-boot + load window so the tensor engine p-state is fully
        # ramped (and stays ramped) when TT starts
        warm_in = const_pool.tile([P, 512], BF16, tag="warmin")
        nc.vector.memset(warm_in[:], 0.001)
        warm = psum.tile([P, Q], F32, tag="ps", name="warm")
        for _ in range(26):
            nc.tensor.matmul(
                warm[:, 0:512],
                lhsT=warm_in[:, 0:P],
                rhs=warm_in[:, 0:512],
                start=True,
                stop=True,
            )
        # TT contraction d=0..3 uses fp8 (m28 + xa8 query cols); only the
        # upper half of m2/xq is needed in bf16
        for g in range(2):
            for i in range(2):
                nc.scalar.dma_start(
                    m28[g][:, i * D : (i + 1) * D],
                    m28_d[(2 * g + i) * P : (2 * g + i + 1) * P, :],
                )
        nc.scalar.dma_start(w2_sb[:], w2_d[:, :])
        # bv broadcast to all partitions (stride-0 DMA source) so the O bias
        # is a plain tensor_add fused into the eviction, not rank-1 matmuls
        nc.scalar.dma_start(bvb_sb[:], bv_d[0:1, :].partition_broadcast(P))
        def load_x8(d):
            g, i = divmod(d, 2)
            nc.sync.dma_start(
                xa8[g][:, i * S : (i + 1) * S], x8_d[d * P : (d + 1) * P, :]
            )

        # d=0..3 of x8 feed TT's DoubleRow half, xq feeds its bf16 half,
        # d=4..7 of x8 are only needed once scores start
        for d in range(ND // 2):
            load_x8(d)
            nc.scalar.dma_start(
                m2[d][:], m2_d[(ND // 2 + d) * P : (ND // 2 + d + 1) * P, :]
            )
        for dd in range(ND // 2):
            nc.sync.dma_start(xa[dd][:], xq_d[dd * P : (dd + 1) * P, :])
        for d in range(ND // 2, ND):
            load_x8(d)
        # xn: [2048,1024] -> two [128, 8*1024] tiles, partition p holds
        # rows {k*128+p}; one strided DMA each
        xn_src = xn_d.ap().rearrange("(i k p) d -> i p k d", i=2, p=P)
        for i in range(2):
            nc.sync.dma_start(
                xn[i].rearrange("p (k d) -> p k d", k=8), xn_src[i]
            )
        for d in range(ND):
            nc.sync.dma_start(wv[d][:], wv_d[d * P : (d + 1) * P, :])

        # ---- TT[d',q] = M2^T xq + w2, evicted to fp8 k-subtile pairs ----
        tt8 = [
            tt_pool.tile([P, 2 * Q], F8, tag="tt", name=f"tt8_{i}")
            for i in range(ND // 2)
        ]
        for eb in range(2):
            pss = [
                psum.tile([P, Q], F32, tag="ps", name=f"tps{eb}_{j}")
                for j in range(4)
            ]
            for g in range(2):
                for j in range(4):
                    e = eb * 4 + j
                    lt = m28[g].rearrange("p (i e) -> p i e", i=2)[
                        :, :, e * P : (e + 1) * P
                    ]
                    for n in range(2):
                        rh = xa8[g].rearrange("p (i s) -> p i s", i=2)[
                            :, :, n * 512 : (n + 1) * 512
                        ]
                        nc.tensor.matmul(
                            pss[j][:, n * 512 : (n + 1) * 512],
                            lhsT=lt,
                            rhs=rh,
                            start=(g == 0),
                            stop=False,
                            perf_mode=mybir.MatmulPerfMode.DoubleRow,
                        )
            for dd in range(ND // 2):
                for j in range(4):
                    e = eb * 4 + j
                    for n in range(2):
                        nc.tensor.matmul(
                            pss[j][:, n * 512 : (n + 1) * 512],
                            lhsT=m2[dd][:, e * P : (e + 1) * P],
                            rhs=xa[dd][:, n * 512 : (n + 1) * 512],
                            start=False,
                            stop=(dd == ND // 2 - 1),
                        )
            for j in range(4):
                e = eb * 4 + j
                dst = tt8[e // 2][:, (e % 2) * Q : (e % 2) * Q + Q]
                if j % 2 == 0:
                    nc.scalar.activation(
                        dst, pss[j][:], AF.Identity, bias=w2_sb[:, e : e + 1]
                    )
                else:
                    nc.vector.tensor_scalar_add(dst, pss[j][:], w2_sb[:, e : e + 1])

        # ---- attention, software-pipelined over 8 q-chunks ----
        def scores_mm(qc):
            """fp8 DoubleRow scores matmuls; returns the two psum halves."""
            pss = []
            for half in range(2):
                ps = psum.tile([P, Q], F32, tag="ps", name=f"sps{qc}_{half}")
                for g in range(ND // 2):
                    lt = tt8[g].rearrange("p (i q) -> p i q", i=2)[
                        :, :, qc * P : (qc + 1) * P
                    ]
                    for n in range(2):
                        off = half * 1024 + n * 512
                        rh = xa8[g].rearrange("p (i s) -> p i s", i=2)[
                            :, :, off : off + 512
                        ]
                        nc.tensor.matmul(
                            ps[:, n * 512 : (n + 1) * 512],
                            lhsT=lt,
                            rhs=rh,
                            start=(g == 0),
                            stop=(g == ND // 2 - 1),
                            perf_mode=mybir.MatmulPerfMode.DoubleRow,
                        )
                pss.append(ps)
            return pss

        def exp_phase(qc, pss):
            """(+mask) + exp + row sums for q-chunk qc."""
            exp_sb = exp_pool.tile([P, S], BF16, tag="exp", name=f"exp{qc}")
            sums = stat_pool.tile([P, 2], F32, tag="sums", name=f"sums{qc}")
            for half in range(2):
                ps = pss[half]
                if use_mask:
                    mt = m_pool.tile([P, Q], F32, tag="m", name=f"mt{qc}_{half}")
                    nc.sync.dma_start(
                        mt[:],
                        mask_d[qc * P : (qc + 1) * P, half * 1024 : (half + 1) * 1024],
                    )
                    nc.vector.tensor_add(ps[:], ps[:], mt[:])
                nc.scalar.activation(
                    exp_sb[:, half * 1024 : (half + 1) * 1024],
                    ps[:],
                    AF.Exp,
                    scale=SCALE,
                    accum_out=sums[:, half : half + 1],
                )
            return exp_sb, sums

        def at_phase(qc, exp_sb, sums):
            """rowsum stats + one xbar transpose of exp for q-chunk qc."""
            rsum = stat_pool.tile([P, 1], F32, tag="rsum", name=f"rsum{qc}")
            nc.vector.tensor_add(rsum[:], sums[:, 0:1], sums[:, 1:2])
            rinv = stat_pool.tile([P, 1], F32, tag="rinv", name=f"rinv{qc}")
            nc.vector.reciprocal(rinv[:], rsum[:])
            at_sb = at_pool.tile([P, S], BF16, tag="at", name=f"at{qc}")
            # xbar transpose in halves: out[p, c, q] = exp[q, c*128+p]; the
            # first half only needs exp h0, so PV' k=0..7 unblocks earlier
            for hh in range(2):
                sl = slice(hh * 1024, (hh + 1) * 1024)
                nc.scalar.dma_start(
                    out=at_sb[:, sl].rearrange("p (c q) -> p c q", q=P),
                    in_=exp_sb[:, sl],
                    transpose=True,
                )
            return at_sb, rinv

        def pv_mm(qc, at_sb, rinv):
            """PV' = A @ x, evicted *1/rowsum to bf16 + transposed."""
            pv = psum.tile([P, D], F32, tag="ps", name=f"pv{qc}")
            for k in range(NS):
                for n in range(2):
                    nc.tensor.matmul(
                        pv[:, n * 512 : (n + 1) * 512],
                        lhsT=at_sb[:, k * P : (k + 1) * P],
                        rhs=xn[k // 8][
                            :, (k % 8) * 1024 + n * 512 : (k % 8) * 1024 + (n + 1) * 512
                        ],
                        start=(k == 0),
                        stop=(k == NS - 1),
                    )
            # evict+transpose in halves so O's first d-chunks unblock after
            # half the latency (the O phase follows only ~3.5us of scores)
            pvn = pvn_pool.tile([P, D], BF16, tag="pvn", name=f"pvn{qc}")
            pvt = pvt_pool.tile([P, D], BF16, tag="pvt", name=f"pvt{qc}")
            for hh in range(2):
                sl = slice(hh * 512, (hh + 1) * 512)
                nc.vector.tensor_scalar_mul(pvn[:, sl], pv[:, sl], rinv[:])
                nc.scalar.dma_start(
                    out=pvt[:, sl].rearrange("p (c q) -> p c q", q=P),
                    in_=pvn[:, sl],
                    transpose=True,
                )
            return pvt

        def o_phase(qc, pvt):
            """O = PV'_norm @ Wv + bv for q-chunk qc, then DMA out."""
            op = psum.tile([P, D], F32, tag="ps", name=f"op{qc}")
            for dc in range(ND):
                for n in range(2):
                    nc.tensor.matmul(
                        op[:, n * 512 : (n + 1) * 512],
                        lhsT=pvt[:, dc * P : (dc + 1) * P],
                        rhs=wv[dc][:, n * 512 : (n + 1) * 512],
                        start=(dc == 0),
                        stop=(dc == ND - 1),
                    )
            oo = o_pool.tile([P, D], BF16, tag="o", name=f"oo{qc}")
            if qc == NQ - 1:
                # final chunk is the kernel tail: evict+store in halves
                for hh in range(2):
                    sl = slice(hh * 512, (hh + 1) * 512)
                    nc.vector.tensor_add(oo[:, sl], op[:, sl], bvb_sb[:, sl])
                    nc.sync.dma_start(
                        out_d[qc * P : (qc + 1) * P, sl], oo[:, sl]
                    )
            else:
                nc.vector.tensor_add(oo[:], op[:], bvb_sb[:, :])
                nc.sync.dma_start(out_d[qc * P : (qc + 1) * P, :], oo[:])

        pend = exp_phase(0, scores_mm(0))
        prev_pvt = None
        for qc in range(NQ):
            if qc + 1 < NQ:
                pss = scores_mm(qc + 1)
                if prev_pvt is not None:
                    o_phase(qc - 1, prev_pvt)
                else:
                    # iteration 0 has no O phase to cover the exp->at-T
                    # latency of chunk 0; keep the PE warm with fillers
                    # (this alloc also keeps the psum ring period-4)
                    warm2 = psum.tile([P, Q], F32, tag="ps", name="warm2")
                    for _ in range(8):
                        nc.tensor.matmul(
                            warm2[:, 0:512],
                            lhsT=warm_in[:, 0:P],
                            rhs=warm_in[:, 0:512],
                            start=True,
                            stop=True,
                        )
                atr = at_phase(qc, *pend)
                pend = exp_phase(qc + 1, pss)
                prev_pvt = pv_mm(qc, *atr)
            else:
                # last chunk: no scores to pipeline, so run PV' first and
                # let O(qc-1) cover the evict+transpose latency of pvt(qc)
                atr = at_phase(qc, *pend)
                cur = pv_mm(qc, *atr)
                o_phase(qc - 1, prev_pvt)
                prev_pvt = cur
        o_phase(NQ - 1, prev_pvt)

    nc.compile()
    return nc


def _get_nc(use_mask: bool) -> bacc.Bacc:
    if use_mask not in _NC_CACHE:
        _NC_CACHE[use_mask] = _build(use_mask)
    return _NC_CACHE[use_mask]


def kernel(x, mask, Wq, bq, Wk, bk, Wv, bv):
    x = np.asarray(x, dtype=np.float32)
    mask = np.asarray(mask, dtype=np.float32)
    Wq = np.asarray(Wq, dtype=np.float32)
    bq = np.asarray(bq, dtype=np.float32)
    Wk = np.asarray(Wk, dtype=np.float32)
    bk = np.asarray(bk, dtype=np.float32)
    Wv = np.asarray(Wv, dtype=np.float32)
    bv = np.asarray(bv, dtype=np.float32)

    B = x.shape[0]
    use_mask = bool(np.any(mask))
    nc = _get_nc(use_mask)

    bf = ml_dtypes.bfloat16
    # scores(i,j) = q_i.k_j - alpha_i with M2 = Wq Wk^T, w2 = Wk bq;
    # alpha_i is a per-row constant that softmax drops.
    m2 = (Wq.astype(np.float64) @ Wk.astype(np.float64).T).astype(bf)
    m28 = m2[0 : D // 2, :].astype(ml_dtypes.float8_e4m3)
    w2 = (Wk.astype(np.float64) @ bq.astype(np.float64)).astype(np.float32)
    w22 = np.ascontiguousarray(w2.reshape(ND, P).T)
    wv_b = Wv.astype(bf)
    bvr = bv.reshape(1, D).astype(bf)

    in_maps = []
    for c in range(8):
        b, h = divmod(c, 2)
        # key order: own query rows first, then the other half
        xp = np.concatenate(
            [x[b, h * Q : (h + 1) * Q], x[b, (1 - h) * Q : (2 - h) * Q]]
        )
        xn = np.ascontiguousarray(xp).astype(bf)
        xpt = np.ascontiguousarray(xp.T)
        im = {
            "xq": np.ascontiguousarray(xpt[D // 2 :, 0:Q]).astype(bf),
            "x8": xpt.astype(ml_dtypes.float8_e4m3),
            "xn": xn,
            "m2": m2,
            "m28": m28,
            "wv": wv_b,
            "w22": w22,
            "bvr": bvr,
        }
        if use_mask:
            mrows = mask[h * Q : (h + 1) * Q]
            mperm = np.concatenate(
                [mrows[:, h * Q : (h + 1) * Q], mrows[:, (1 - h) * Q : (2 - h) * Q]],
                axis=1,
            )
            im["maskp"] = np.ascontiguousarray(mperm / np.float32(SCALE)).astype(
                np.float32
            )
        in_maps.append(im)

    res = run_bass_kernel_spmd(nc, in_maps, core_ids=list(range(8)))

    out = np.empty((B, S, D), dtype=np.float32)
    for c in range(8):
        b, h = divmod(c, 2)
        out[b, h * Q : (h + 1) * Q, :] = res.results[c]["out"].astype(np.float32)
    return out
